# revision 1
# baseline (speedup 1.0000x reference)
"""ArcFace (AngularPenaltySMLoss) over [32768, 8192] f32, distributed over
8 TRN2 NeuronCores, data-parallel on the batch dim.

Per core: shard [4096, 8192]. For each 128-row tile:
  - DMA tile to SBUF (sync-engine HWDGE ring; one ring keeps tile
    completions sequential so the double-buffer never stalls)
  - ScalarE: exp(S*x) with fused free-dim accumulation -> row exp-sums
  - VectorE: scalar_tensor_tensor (iota == label) * x with fused free-dim
    accumulation -> gathers target = x[row, label] (one nonzero per row)
Epilogue (two batches; the first hides inside the loop):
  numerator = S*(t*cos(M) - sin(M)*sqrt(1 - t^2))   # = S*cos(acos(t)+M)
  with sqrt(y) computed as exp(0.5*ln(y)) so the only ACT table set used
  anywhere is natural_log_exp (zero mid-kernel table switches).
  L = numerator - log(exp(numerator) + rowsum - exp(S*t))
  partial = sum(L) per core -> [128,1]; host sums, loss = -total/N.
"""

import numpy as np

from concourse import bacc, hw_specs, mybir, tile
from concourse.bass_utils import run_bass_kernel_spmd

# The act-table placement pass picks the FIRST set containing each
# activation function, so an Exp/Ln mix thrashes between exp_and_others and
# natural_log (8 table loads here, 3 on the critical tail). Present a view
# of the tables with Exp/Ln stripped from every set except the combined
# natural_log_exp_and_others so both resolve to one set (one load total).
# Only membership changes; set order/ids still match act_info.json.
_ORIG_GET_TABLES = hw_specs.get_activation_tables
_COMBINED_SET = "natural_log_exp_and_others"


def _exp_ln_combined_tables(arch):
    tabs = _ORIG_GET_TABLES(arch)
    AF = mybir.ActivationFunctionType
    if _COMBINED_SET not in tabs:
        return tabs
    return {
        name: (fns - {AF.Exp, AF.Ln} if name != _COMBINED_SET else fns)
        for name, fns in tabs.items()
    }



N, C = 32768, 8192
N_CORES = 8
N_SHARD = N // N_CORES      # 4096 rows per core
P = 128                     # SBUF partitions
N_TILES = N_SHARD // P      # 32 tiles per core
S = 32.0
M = 0.5
EPS = 1e-7

_F32 = mybir.dt.float32


def build(n_shard=N_SHARD, c=C, dual_ring=False, psum_et=True, gp_cols=0):
    # gp_cols>0 (offloading part of the gather to GpSimd's
    # scalar_tensor_tensor) fails to compile in this backend — keep 0.
    # dual_ring=True (alternating x-tile DMAs across the SP and ACT HWDGE
    # rings) measured consistently ~40us SLOWER in interleaved A/B: the two
    # rings' transfers share the 16 SDMA engines, so tile k's completion is
    # delayed by tile k+1's concurrent transfer, stalling the double-buffer.
    prev_tables = bacc.get_activation_tables
    bacc.get_activation_tables = _exp_ln_combined_tables
    try:
        return _build(n_shard, c, dual_ring, psum_et, gp_cols)
    finally:
        bacc.get_activation_tables = prev_tables


def _build(n_shard, c, dual_ring, psum_et, gp_cols):
    n_tiles = n_shard // P
    nc = bacc.Bacc(None, target_bir_lowering=False)

    x_ext = nc.declare_dram_parameter("cls_score", [n_shard, c], _F32, isOutput=False)
    lab_ext = nc.declare_dram_parameter("labels_t", [P, n_tiles], _F32, isOutput=False)
    out_ext = nc.declare_dram_parameter("out", [P, 1], _F32, isOutput=True)

    AF = mybir.ActivationFunctionType
    OP = mybir.AluOpType
    AX = mybir.AxisListType

    split = n_tiles - 1 if n_tiles > 1 else 1

    # et (the exp output) is write-only scratch. With psum_et it goes to
    # PSUM (ScalarE's faster port) in two 4096-wide chunks (PSUM free-dim
    # cap). The gather also runs in two half-width chunks (A/B accumulators
    # folded in the epilogue), shrinking its scratch to half a tile; the
    # freed SBUF deepens the x stream to 4 buffers.
    x_bufs = 4 if psum_et else 2
    half = c // 2 if c > 1 else 1

    with tile.TileContext(nc) as tc:
        with (
            tc.tile_pool(name="xp", bufs=x_bufs) as xp,
            tc.tile_pool(name="ep", bufs=1,
                         space="PSUM" if psum_et else "SBUF") as ep,
            tc.tile_pool(name="mp", bufs=1) as mp,
            tc.tile_pool(name="st", bufs=1) as st,
        ):
            lab = st.tile([P, n_tiles], _F32)
            nc.scalar.dma_start(out=lab[:], in_=lab_ext[:])
            iota = st.tile([P, c], _F32)  # each row = [0..c-1]
            nc.gpsimd.iota(iota[:], pattern=[[1, c]], base=0,
                           channel_multiplier=0,
                           allow_small_or_imprecise_dtypes=True)

            sumexp = st.tile([P, n_tiles], _F32)
            sumexpA = st.tile([P, n_tiles], _F32)  # first-half chunk sums
            sumexpB = st.tile([P, n_tiles], _F32)  # second-half chunk sums
            tvals = st.tile([P, n_tiles], _F32)
            # gather accumulators per column range; the range without the
            # label sums to exactly 0, so tvals = A + B
            tvalsA = st.tile([P, n_tiles], _F32)
            tvalsB = st.tile([P, n_tiles], _F32)

            # epilogue scratch, written in column batches
            tclip = st.tile([P, n_tiles], _F32)
            tsq = st.tile([P, n_tiles], _F32)
            om = st.tile([P, n_tiles], _F32)
            lnom = st.tile([P, n_tiles], _F32)
            r = st.tile([P, n_tiles], _F32)
            b_t = st.tile([P, n_tiles], _F32)
            num = st.tile([P, n_tiles], _F32)
            e_num = st.tile([P, n_tiles], _F32)
            e_st = st.tile([P, n_tiles], _F32)
            excl = st.tile([P, n_tiles], _F32)
            denom = st.tile([P, n_tiles], _F32)
            logd = st.tile([P, n_tiles], _F32)
            ell = st.tile([P, n_tiles], _F32)

            def epilogue(sl):
                # all [P, width] ops; only Exp/Ln on ACT (one table set)
                if psum_et:
                    nc.vector.tensor_tensor(
                        sumexp[:, sl], sumexpA[:, sl], sumexpB[:, sl], OP.add)
                nc.vector.tensor_tensor(
                    tvals[:, sl], tvalsA[:, sl], tvalsB[:, sl], OP.add)
                nc.vector.tensor_scalar(
                    tclip[:, sl], tvals[:, sl], -1.0 + EPS, 1.0 - EPS,
                    OP.max, OP.min)
                nc.vector.tensor_tensor(tsq[:, sl], tclip[:, sl], tclip[:, sl],
                                        OP.mult)
                nc.vector.tensor_scalar(om[:, sl], tsq[:, sl], -1.0, 1.0,
                                        OP.mult, OP.add)  # 1 - t^2
                nc.scalar.activation(out=lnom[:, sl], in_=om[:, sl], func=AF.Ln)
                nc.scalar.activation(out=r[:, sl], in_=lnom[:, sl], func=AF.Exp,
                                     scale=0.5)  # sqrt(1-t^2)
                nc.vector.tensor_scalar_mul(b_t[:, sl], r[:, sl],
                                            S * float(np.sin(M)))
                nc.vector.scalar_tensor_tensor(
                    num[:, sl], tclip[:, sl], S * float(np.cos(M)), b_t[:, sl],
                    OP.mult, OP.subtract)
                nc.scalar.activation(out=e_num[:, sl], in_=num[:, sl], func=AF.Exp)
                nc.scalar.activation(out=e_st[:, sl], in_=tvals[:, sl],
                                     func=AF.Exp, scale=S)
                nc.vector.scalar_tensor_tensor(
                    excl[:, sl], e_st[:, sl], -1.0, sumexp[:, sl],
                    OP.mult, OP.add)  # sumexp - exp(S t)
                nc.vector.tensor_tensor(denom[:, sl], excl[:, sl], e_num[:, sl],
                                        OP.add)
                nc.scalar.activation(out=logd[:, sl], in_=denom[:, sl], func=AF.Ln)
                nc.vector.tensor_tensor(ell[:, sl], num[:, sl], logd[:, sl],
                                        OP.subtract)

            halves = (slice(0, half), slice(half, c))
            acc_cols = (sumexpA, sumexpB)

            def do_exp(xt, k, col_off=0):
                # exp(S*x) + row-sum; two PSUM-sized chunks when psum_et
                if psum_et:
                    for h, cs in enumerate(halves):
                        et = ep.tile([P, half], _F32)
                        nc.scalar.activation(
                            out=et[:], in_=xt[:, cs], func=AF.Exp, scale=S,
                            accum_out=acc_cols[h][:, k:k + 1],
                        )
                else:
                    et = ep.tile([P, c], _F32)
                    nc.scalar.activation(
                        out=et[:], in_=xt[:], func=AF.Exp, scale=S,
                        accum_out=sumexp[:, k:k + 1],
                    )

            for k in range(n_tiles):
                ring = nc.scalar if (dual_ring and k % 2) else nc.sync
                if k < n_tiles - 1 or n_tiles == 1:
                    xt = xp.tile([P, c], _F32)
                    ring.dma_start(out=xt[:], in_=x_ext[k * P:(k + 1) * P, :])
                    do_exp(xt, k)
                    # (iota == label) * x ; accum -> x[row, label], in two
                    # half-width chunks so the scratch is half a tile
                    t_acc = (tvalsA, tvalsB)
                    for h, cs in enumerate(halves):
                        mt = mp.tile([P, half], _F32)
                        nc.vector.scalar_tensor_tensor(
                            mt[:], iota[:, cs], lab[:, k:k + 1], xt[:, cs],
                            OP.is_equal, OP.mult,
                            accum_out=t_acc[h][:, k:k + 1],
                        )
                else:
                    # last tile in column halves: only ~half a tile of
                    # compute remains exposed after the final transfer
                    t_acc = (tvalsA, tvalsB)
                    for h, cs in enumerate(halves):
                        xt = xp.tile([P, half], _F32)
                        (nc.scalar if (dual_ring and h % 2) else nc.sync).dma_start(
                            out=xt[:], in_=x_ext[k * P:(k + 1) * P, cs])
                        et = ep.tile([P, half], _F32)
                        nc.scalar.activation(
                            out=et[:], in_=xt[:], func=AF.Exp, scale=S,
                            accum_out=acc_cols[h][:, k:k + 1],
                        )
                        mt = mp.tile([P, half], _F32)
                        # iota slice keeps global column indices
                        nc.vector.scalar_tensor_tensor(
                            mt[:], iota[:, cs], lab[:, k:k + 1], xt[:],
                            OP.is_equal, OP.mult,
                            accum_out=t_acc[h][:, k:k + 1],
                        )
                    if not psum_et:
                        nc.vector.tensor_tensor(
                            sumexp[:, k:k + 1], sumexpA[:, k:k + 1],
                            sumexpB[:, k:k + 1], OP.add)
                if k == split - 1 and n_tiles > 1:
                    epilogue(slice(0, split))

            epilogue(slice(split, n_tiles) if n_tiles > 1 else slice(0, n_tiles))

            lrow = st.tile([P, 1], _F32)
            nc.vector.tensor_reduce(lrow[:], ell[:], axis=AX.X, op=OP.add)
            nc.scalar.dma_start(out=out_ext[:], in_=lrow[:])

    nc.finalize()
    return nc


_NC_CACHE = {}


def _get_nc():
    if "nc" not in _NC_CACHE:
        _NC_CACHE["nc"] = build()
    return _NC_CACHE["nc"]


def make_in_maps(cls_score, labels):
    cls_score = np.ascontiguousarray(np.asarray(cls_score, dtype=np.float32))
    labels = np.asarray(labels).astype(np.int64)
    in_maps = []
    for i in range(N_CORES):
        shard = cls_score[i * N_SHARD:(i + 1) * N_SHARD]
        lab_i = labels[i * N_SHARD:(i + 1) * N_SHARD].astype(np.float32)
        # [n_tiles, P] -> [P, n_tiles]: partition p, col k = label of row k*P+p
        lab_t = np.ascontiguousarray(lab_i.reshape(N_TILES, P).T)
        in_maps.append({"cls_score": shard, "labels_t": lab_t})
    return in_maps


def kernel(cls_score, labels):
    nc = _get_nc()
    in_maps = make_in_maps(cls_score, labels)
    res = run_bass_kernel_spmd(nc, in_maps, core_ids=list(range(N_CORES)))
    total = np.sum(
        [r["out"].astype(np.float64).sum() for r in res.results]
    )
    return np.float32(-(total / N))



# revision 2
# speedup vs baseline: 1.0198x; 1.0198x over previous
"""ArcFace (AngularPenaltySMLoss) over [32768, 8192] f32, distributed over
8 TRN2 NeuronCores, data-parallel on the batch dim.

The kernel is HBM-bound: every byte of cls_score must be read once
(128 MiB/core), and the SDMA stream already runs gap-free at ~345 GB/s —
the per-NC share of the 716 GB/s HBM stack. So the device does ONLY the
memory-bound part: stream the shard and compute per-row sums of
exp(S*x) via ScalarE activation with fused free-dim accumulation (exp
output dumped to PSUM, two 4096-col chunks per tile = PSUM capacity).
The O(N) epilogue (target gather, arcface numerator, log) runs on host
in float64 off the [N] row exp-sums — this removes the VectorE
iota==label gather (which ran at ~97 G elem/s, nearly DMA-rate), the
GpSimd iota (16.5 us), the labels DMA, and the on-device epilogue+log
table loads, all of which sat on the baseline's 33 us tail.

Tile layout per core: 32 row-tiles of [128, 8192]. Tiles 0..30 stream
as whole-tile 4 MiB DMAs on the sync HWDGE ring (one ring keeps tile
completions sequential; dual-ring measured slower). Tile 31 streams as
8 separate 1024-col chunk DMAs so the compute exposed after the final
DMA is one ~1.2 us exp chunk instead of a 4096-col half-tile. Row-sum
partials go out as [128, 70] f32 in two DMAs on the scalar ring: cols
0..61 (tiles 0..30) issued early so their completion hides under the
tail stream, cols 62..69 (tile 31 chunks) at the end.
"""

import numpy as np

from concourse import bacc, mybir, tile
from concourse.bass_utils import run_bass_kernel_spmd

N, C = 32768, 8192
N_CORES = 8
N_SHARD = N // N_CORES      # 4096 rows per core
P = 128                     # SBUF partitions
N_TILES = N_SHARD // P      # 32 tiles per core
S = 32.0
M = 0.5
EPS = 1e-7

_F32 = mybir.dt.float32

HALF = C // 2               # 4096-wide exp chunks (PSUM free-dim cap)
TAIL_CHUNK = 1024           # last tile streams in 8 column chunks
TAIL_CHUNKS = C // TAIL_CHUNK
MAIN_COLS = 2 * (N_TILES - 1)        # 62 accum cols for tiles 0..30
OUT_COLS = MAIN_COLS + TAIL_CHUNKS   # 70


def build(n_shard=N_SHARD, c=C):
    n_tiles = n_shard // P
    main_cols = 2 * (n_tiles - 1)
    nc = bacc.Bacc(None, target_bir_lowering=False)

    x_ext = nc.declare_dram_parameter("cls_score", [n_shard, c], _F32, isOutput=False)
    out_ext = nc.declare_dram_parameter(
        "out", [P, main_cols + TAIL_CHUNKS], _F32, isOutput=True)

    AF = mybir.ActivationFunctionType
    half = c // 2

    with tile.TileContext(nc) as tc:
        with (
            tc.tile_pool(name="xp", bufs=4) as xp,
            tc.tile_pool(name="ep", bufs=1, space="PSUM") as ep,
            tc.tile_pool(name="st", bufs=1) as st,
        ):
            # separate main/tail accumulator tiles so the early out DMA
            # (reads main) has no WAR hazard against the tail accums
            sums_main = st.tile([P, main_cols], _F32)
            sums_tail = st.tile([P, TAIL_CHUNKS], _F32)

            for k in range(n_tiles - 1):
                xt = xp.tile([P, c], _F32)
                nc.sync.dma_start(out=xt[:], in_=x_ext[k * P:(k + 1) * P, :])
                for h in range(2):
                    et = ep.tile([P, half], _F32)
                    nc.scalar.activation(
                        out=et[:], in_=xt[:, h * half:(h + 1) * half],
                        func=AF.Exp, scale=S,
                        accum_out=sums_main[:, 2 * k + h:2 * k + h + 1])

            # main row-sum partials out; completes under the tail stream
            nc.scalar.dma_start(out=out_ext[:, :main_cols], in_=sums_main[:])

            k = n_tiles - 1
            for j in range(TAIL_CHUNKS):
                xt = xp.tile([P, TAIL_CHUNK], _F32)
                nc.sync.dma_start(
                    out=xt[:],
                    in_=x_ext[k * P:(k + 1) * P,
                              j * TAIL_CHUNK:(j + 1) * TAIL_CHUNK])
                et = ep.tile([P, TAIL_CHUNK], _F32)
                nc.scalar.activation(
                    out=et[:], in_=xt[:], func=AF.Exp, scale=S,
                    accum_out=sums_tail[:, j:j + 1])

            nc.scalar.dma_start(out=out_ext[:, main_cols:], in_=sums_tail[:])

    nc.finalize()
    return nc


_NC_CACHE = {}


def _get_nc():
    if "nc" not in _NC_CACHE:
        _NC_CACHE["nc"] = build()
    return _NC_CACHE["nc"]


def make_in_maps(cls_score):
    cls_score = np.ascontiguousarray(np.asarray(cls_score, dtype=np.float32))
    return [
        {"cls_score": cls_score[i * N_SHARD:(i + 1) * N_SHARD]}
        for i in range(N_CORES)
    ]


def postprocess(results, cls_score, labels):
    """Host epilogue in float64 off the device per-row exp-sums."""
    cls_score = np.asarray(cls_score, dtype=np.float32)
    labels = np.asarray(labels).astype(np.int64)
    rowsum = np.empty((N,), dtype=np.float64)
    for i, r in enumerate(results):
        o = r["out"].astype(np.float64)                    # [P, OUT_COLS]
        main = o[:, :MAIN_COLS].reshape(P, N_TILES - 1, 2).sum(axis=2)
        tailv = o[:, MAIN_COLS:].sum(axis=1)               # [P]
        # shard row n = k*P + p  ->  main[p, k] (k < 31) or tailv[p]
        rs = np.concatenate([main.T.reshape(-1), tailv])
        rowsum[i * N_SHARD:(i + 1) * N_SHARD] = rs
    target = cls_score[np.arange(N), labels].astype(np.float64)
    t = np.clip(target, -1.0 + EPS, 1.0 - EPS)
    num = S * np.cos(np.arccos(t) + M)
    excl = rowsum - np.exp(S * target)
    L = num - np.log(np.exp(num) + excl)
    return np.float32(-np.mean(L))


def kernel(cls_score, labels):
    nc = _get_nc()
    in_maps = make_in_maps(cls_score)
    res = run_bass_kernel_spmd(nc, in_maps, core_ids=list(range(N_CORES)))
    return postprocess(res.results, cls_score, labels)


# revision 3
# speedup vs baseline: 1.2125x; 1.1890x over previous
"""ArcFace (AngularPenaltySMLoss) over [32768, 8192] f32, distributed over
8 TRN2 NeuronCores, data-parallel on the batch dim.

The kernel is HBM-bound: every byte of cls_score must be read once
(128 MiB/core), and the SDMA stream already runs gap-free at ~345 GB/s —
the per-NC share of the 716 GB/s HBM stack. So the device does ONLY the
memory-bound part: stream the shard and compute per-row sums of
exp(S*x) via ScalarE activation with fused free-dim accumulation (exp
output dumped to PSUM, two 4096-col chunks per tile = PSUM capacity).
The O(N) epilogue (target gather, arcface numerator, log) runs on host
in float64 off the [N] row exp-sums — this removes the VectorE
iota==label gather (which ran at ~97 G elem/s, nearly DMA-rate), the
GpSimd iota (16.5 us), the labels DMA, and the on-device epilogue+log
table loads, all of which sat on the baseline's 33 us tail.

Tile layout per core: 32 row-tiles of [128, 8192]. Tiles 0..30 stream
as whole-tile 4 MiB DMAs on the sync HWDGE ring (one ring keeps tile
completions sequential; dual-ring measured slower). Tile 31 streams as
8 separate 1024-col chunk DMAs so the compute exposed after the final
DMA is one ~1.2 us exp chunk instead of a 4096-col half-tile. Row-sum
partials go out as [128, 70] f32 in two DMAs on the scalar ring: cols
0..61 (tiles 0..30) issued early so their completion hides under the
tail stream, cols 62..69 (tile 31 chunks) at the end.
"""

import numpy as np

from concourse import bacc, mybir, tile
from concourse.bass_utils import run_bass_kernel_spmd

N, C = 32768, 8192
N_CORES = 8
N_SHARD = N // N_CORES      # 4096 rows per core
P = 128                     # SBUF partitions
N_TILES = N_SHARD // P      # 32 tiles per core
S = 32.0
M = 0.5
EPS = 1e-7

_F32 = mybir.dt.float32

HALF = C // 2               # 4096-wide exp chunks (PSUM free-dim cap)
TAIL_CHUNK = 1024           # last tile streams in 8 column chunks
TAIL_CHUNKS = C // TAIL_CHUNK
MAIN_COLS = 2 * (N_TILES - 1)        # 62 accum cols for tiles 0..30
OUT_COLS = MAIN_COLS + TAIL_CHUNKS   # 70


def build(n_shard=N_SHARD, c=C):
    n_tiles = n_shard // P
    main_cols = 2 * (n_tiles - 1)
    nc = bacc.Bacc(None, target_bir_lowering=False)

    x_ext = nc.declare_dram_parameter("cls_score", [n_shard, c], _F32, isOutput=False)
    out_ext = nc.declare_dram_parameter(
        "out", [P, main_cols + TAIL_CHUNKS], _F32, isOutput=True)

    AF = mybir.ActivationFunctionType
    half = c // 2

    with tile.TileContext(nc) as tc:
        with (
            tc.tile_pool(name="xp", bufs=6) as xp,
            tc.tile_pool(name="ep", bufs=1, space="PSUM") as ep,
            tc.tile_pool(name="st", bufs=1) as st,
        ):
            # separate main/tail accumulator tiles so the early out DMA
            # (reads main) has no WAR hazard against the tail accums
            sums_main = st.tile([P, main_cols], _F32)
            sums_tail = st.tile([P, TAIL_CHUNKS], _F32)

            for k in range(n_tiles - 1):
                xt = xp.tile([P, c], _F32)
                nc.sync.dma_start(out=xt[:], in_=x_ext[k * P:(k + 1) * P, :])
                for h in range(2):
                    et = ep.tile([P, half], _F32)
                    nc.scalar.activation(
                        out=et[:], in_=xt[:, h * half:(h + 1) * half],
                        func=AF.Exp, scale=S,
                        accum_out=sums_main[:, 2 * k + h:2 * k + h + 1])

            # main row-sum partials out; completes under the tail stream
            nc.scalar.dma_start(out=out_ext[:, :main_cols], in_=sums_main[:])

            k = n_tiles - 1
            for j in range(TAIL_CHUNKS):
                xt = xp.tile([P, TAIL_CHUNK], _F32)
                nc.sync.dma_start(
                    out=xt[:],
                    in_=x_ext[k * P:(k + 1) * P,
                              j * TAIL_CHUNK:(j + 1) * TAIL_CHUNK])
                et = ep.tile([P, TAIL_CHUNK], _F32)
                nc.scalar.activation(
                    out=et[:], in_=xt[:], func=AF.Exp, scale=S,
                    accum_out=sums_tail[:, j:j + 1])

            nc.scalar.dma_start(out=out_ext[:, main_cols:], in_=sums_tail[:])

    nc.finalize()
    return nc


_NC_CACHE = {}


def _get_nc():
    if "nc" not in _NC_CACHE:
        _NC_CACHE["nc"] = build()
    return _NC_CACHE["nc"]


def make_in_maps(cls_score):
    cls_score = np.ascontiguousarray(np.asarray(cls_score, dtype=np.float32))
    return [
        {"cls_score": cls_score[i * N_SHARD:(i + 1) * N_SHARD]}
        for i in range(N_CORES)
    ]


def postprocess(results, cls_score, labels):
    """Host epilogue in float64 off the device per-row exp-sums."""
    cls_score = np.asarray(cls_score, dtype=np.float32)
    labels = np.asarray(labels).astype(np.int64)
    rowsum = np.empty((N,), dtype=np.float64)
    for i, r in enumerate(results):
        o = r["out"].astype(np.float64)                    # [P, OUT_COLS]
        main = o[:, :MAIN_COLS].reshape(P, N_TILES - 1, 2).sum(axis=2)
        tailv = o[:, MAIN_COLS:].sum(axis=1)               # [P]
        # shard row n = k*P + p  ->  main[p, k] (k < 31) or tailv[p]
        rs = np.concatenate([main.T.reshape(-1), tailv])
        rowsum[i * N_SHARD:(i + 1) * N_SHARD] = rs
    target = cls_score[np.arange(N), labels].astype(np.float64)
    t = np.clip(target, -1.0 + EPS, 1.0 - EPS)
    num = S * np.cos(np.arccos(t) + M)
    excl = rowsum - np.exp(S * target)
    L = num - np.log(np.exp(num) + excl)
    return np.float32(-np.mean(L))


def kernel(cls_score, labels):
    nc = _get_nc()
    in_maps = make_in_maps(cls_score)
    res = run_bass_kernel_spmd(nc, in_maps, core_ids=list(range(N_CORES)))
    return postprocess(res.results, cls_score, labels)


# revision 4
# speedup vs baseline: 1.2167x; 1.0034x over previous
"""ArcFace (AngularPenaltySMLoss) over [32768, 8192] f32, distributed over
8 TRN2 NeuronCores, data-parallel on the batch dim.

The kernel is DMA-bound: every byte of cls_score must be read once
(128 MiB/core). The device does ONLY the memory-bound part: stream the
shard and compute per-row sums of exp(S*x) via ScalarE activation with
fused free-dim accumulation (exp output dumped to PSUM, two 4096-col
chunks per tile = PSUM capacity). The O(N) epilogue (target gather,
arcface numerator, log) runs on host in float64 off the [N] row
exp-sums — this removes the VectorE iota==label gather (which ran at
~97 G elem/s, nearly DMA rate), the GpSimd iota, the labels DMA, and
the on-device epilogue, all of which sat on the baseline's 33 us tail.

Pipeline facts this layout is built around (measured via NTFF traces):
  - With enough x buffers the 16-SDMA stream runs gap-free at ~430 GB/s
    (SBUF-fabric-bound, 32 KiB descriptors).
  - A DMA's completion SEMAPHORE lags its last data packet by up to
    ~15-30 us under load (write-receipt backlog), so anything gated on
    end-of-stream completions pays that lag. Buffer depth must cover
    lag + the ACT chain (~8.5 us/tile) or the issue loop serializes at
    ~12.2 us/tile (the old 4-buffer kernels' pacing).
  - ScalarE consumes a tile in 2x(4096-col exp + accum-read) = ~8.2 us
    < the ~9.7 us tile stream time, so ACT tracks the stream.

Layout per core: 32 row-tiles of [128, 8192]. Tiles 30 and 31 are
PREFETCHED at kernel start on the scalar HWDGE ring into dedicated
buffers — their data and completion sems arrive minutes early in
pipeline terms, so the endgame after tile 29's exp chain is pure ACT
work with zero DMA waits (no lagged completions on the critical tail).
Tiles 0..29 stream as whole-tile 4 MiB DMAs on the sync ring through a
4-deep pool. Row-sum partials go out as [128, 64] f32 in two DMAs on
the scalar ring: cols 0..59 (tiles 0..29) issued as soon as they are
complete (hides under the endgame), cols 60..63 (tiles 30, 31) at the
end (16 B/partition, ~2 us receipt on a drained queue).
"""

import numpy as np

from concourse import bacc, mybir, tile
from concourse.bass_utils import run_bass_kernel_spmd

N, C = 32768, 8192
N_CORES = 8
N_SHARD = N // N_CORES      # 4096 rows per core
P = 128                     # SBUF partitions
N_TILES = N_SHARD // P      # 32 tiles per core
S = 32.0
M = 0.5
EPS = 1e-7

_F32 = mybir.dt.float32

N_PRE = 2                        # tiles prefetched for the endgame
N_STREAM = N_TILES - N_PRE       # 30 streamed tiles
MAIN_COLS = 2 * N_STREAM         # 60
OUT_COLS = MAIN_COLS + 2 * N_PRE  # 64


def build():
    nc = bacc.Bacc(None, target_bir_lowering=False)

    x_ext = nc.declare_dram_parameter("cls_score", [N_SHARD, C], _F32, isOutput=False)
    out_ext = nc.declare_dram_parameter("out", [P, OUT_COLS], _F32, isOutput=True)

    AF = mybir.ActivationFunctionType
    half = C // 2

    with tile.TileContext(nc) as tc:
        with (
            tc.tile_pool(name="xp", bufs=4) as xp,
            tc.tile_pool(name="pp", bufs=N_PRE) as pp,
            tc.tile_pool(name="ep", bufs=1, space="PSUM") as ep,
            tc.tile_pool(name="st", bufs=1) as st,
        ):
            sums_main = st.tile([P, MAIN_COLS], _F32)
            sums_tail = st.tile([P, 2 * N_PRE], _F32)

            # prefetch the endgame tiles on the scalar ring; data lands
            # within the first few tile-times, completion sems while the
            # receipt queue is still shallow
            pre = []
            for i in range(N_PRE):
                k = N_STREAM + i
                pt = pp.tile([P, C], _F32)
                nc.scalar.dma_start(out=pt[:], in_=x_ext[k * P:(k + 1) * P, :])
                pre.append(pt)

            def exp_tile(xt, acc, col):
                for h in range(2):
                    et = ep.tile([P, half], _F32)
                    nc.scalar.activation(
                        out=et[:], in_=xt[:, h * half:(h + 1) * half],
                        func=AF.Exp, scale=S,
                        accum_out=acc[:, col + h:col + h + 1])

            for k in range(N_STREAM):
                xt = xp.tile([P, C], _F32)
                nc.sync.dma_start(out=xt[:], in_=x_ext[k * P:(k + 1) * P, :])
                exp_tile(xt, sums_main, 2 * k)

            # main partials out; completes under the endgame ACT work
            nc.scalar.dma_start(out=out_ext[:, :MAIN_COLS], in_=sums_main[:])

            for i in range(N_PRE):
                exp_tile(pre[i], sums_tail, 2 * i)

            nc.scalar.dma_start(out=out_ext[:, MAIN_COLS:], in_=sums_tail[:])

    nc.finalize()
    return nc


_NC_CACHE = {}


def _get_nc():
    if "nc" not in _NC_CACHE:
        _NC_CACHE["nc"] = build()
    return _NC_CACHE["nc"]


def make_in_maps(cls_score):
    cls_score = np.ascontiguousarray(np.asarray(cls_score, dtype=np.float32))
    return [
        {"cls_score": cls_score[i * N_SHARD:(i + 1) * N_SHARD]}
        for i in range(N_CORES)
    ]


def postprocess(results, cls_score, labels):
    """Host epilogue in float64 off the device per-row exp-sums."""
    cls_score = np.asarray(cls_score, dtype=np.float32)
    labels = np.asarray(labels).astype(np.int64)
    rowsum = np.empty((N,), dtype=np.float64)
    for i, r in enumerate(results):
        o = r["out"].astype(np.float64)                    # [P, OUT_COLS]
        per_tile = np.concatenate(
            [o[:, :MAIN_COLS].reshape(P, N_STREAM, 2).sum(axis=2),
             o[:, MAIN_COLS:].reshape(P, N_PRE, 2).sum(axis=2)],
            axis=1)                                        # [P, N_TILES]
        # shard row n = k*P + p  ->  per_tile[p, k]
        rowsum[i * N_SHARD:(i + 1) * N_SHARD] = per_tile.T.reshape(-1)
    target = cls_score[np.arange(N), labels].astype(np.float64)
    t = np.clip(target, -1.0 + EPS, 1.0 - EPS)
    num = S * np.cos(np.arccos(t) + M)
    excl = rowsum - np.exp(S * target)
    L = num - np.log(np.exp(num) + excl)
    return np.float32(-np.mean(L))


def kernel(cls_score, labels):
    nc = _get_nc()
    in_maps = make_in_maps(cls_score)
    res = run_bass_kernel_spmd(nc, in_maps, core_ids=list(range(N_CORES)))
    return postprocess(res.results, cls_score, labels)


# revision 12
# speedup vs baseline: 1.2602x; 1.0358x over previous
"""ArcFace (AngularPenaltySMLoss) over [32768, 8192] f32, distributed over
8 TRN2 NeuronCores, data-parallel on the batch dim.

The kernel is DMA-bound: every byte of cls_score must be read once
(128 MiB/core). The device does ONLY the memory-bound part: stream the
shard and compute per-row sums of exp(S*x) via ScalarE activation with
fused free-dim accumulation. The exp output itself is throwaway: it is
dumped to SBUF as saturating fp8e4 so a whole [128, 8192] tile fits one
ACTIVATE (8 KiB/partition; sub-f32 writes to PSUM are rejected by
neuronxcc — matmul/memset only). The accumulator taps pre-cast fp32 —
the loss is bit-identical to the f32-dump variant (rel err 1.6e-7
against the jax reference either way). The O(N) epilogue (target gather,
arcface numerator, log) runs on host in float64 off the [N] row
exp-sums — this removes the VectorE iota==label gather (which ran at
~97 G elem/s, nearly DMA rate), the GpSimd iota, the labels DMA, and
the on-device epilogue, all of which sat on the old 33 us tail.

Pipeline facts this layout is built around (measured via NTFF traces):
  - With enough x buffers the 16-SDMA stream runs gap-free at up to
    ~430 GB/s (SBUF-fabric-bound); on congested-HBM runs packets slow
    to ~340 GB/s, which no kernel structure can beat.
  - A DMA's completion SEMAPHORE lags its last data packet by ~15-35 us
    under load (write-receipt backlog). The x-buffer pool must cover
    lag + the ACT chain (~8.5 us/tile) + issue latency or the issue
    loop serializes at ~12.2+ us/tile: 4- and 5-buffer variants fell
    into that mode on real runs (417-434 us); 6 buffers never did.
  - Whole-tile DMAs maximize the 8-semaphore-lane reuse window
    (8 x 4 MiB in flight); finer-grained streams hit the lane-reuse
    guard earlier and serialize sooner in bad weather.
  - ScalarE consumes a tile in one 8192-col exp + accum-read = ~7.4 us
    (measured 7120 ns ACTIVATE, no SBUF-write throttle) < the ~9.7 us
    tile stream time, so ACT tracks the stream.

Layout per core: 32 row-tiles of [128, 8192] through a 6-deep pool on
the sync HWDGE ring. The last tile streams as 4 separate 2048-col
chunk DMAs so the compute exposed after the final DMA completion is
one ~2.3 us exp chunk instead of a whole-tile chain. Row-sum partials
go out as [128, 35] f32 in two DMAs on the otherwise-idle scalar ring:
cols 0..30 (tiles 0..30) issued so their completion hides under the
endgame, cols 31..34 (last-tile chunks) at the end (16 B/partition,
~2 us receipt on a drained queue).
"""

import numpy as np

from concourse import bacc, mybir, tile
from concourse.bass_utils import run_bass_kernel_spmd

N, C = 32768, 8192
N_CORES = 8
N_SHARD = N // N_CORES      # 4096 rows per core
P = 128                     # SBUF partitions
N_TILES = N_SHARD // P      # 32 tiles per core
S = 32.0
M = 0.5
EPS = 1e-7

_F32 = mybir.dt.float32
_F8 = mybir.dt.float8e4

TAIL_CHUNK = 2048           # last tile streams in 4 column chunks
TAIL_CHUNKS = C // TAIL_CHUNK        # 4
MAIN_COLS = N_TILES - 1              # 31 accum cols for tiles 0..30
OUT_COLS = MAIN_COLS + TAIL_CHUNKS   # 35


def build():
    nc = bacc.Bacc(None, target_bir_lowering=False)

    x_ext = nc.declare_dram_parameter("cls_score", [N_SHARD, C], _F32, isOutput=False)
    out_ext = nc.declare_dram_parameter("out", [P, OUT_COLS], _F32, isOutput=True)

    AF = mybir.ActivationFunctionType
    half = C // 2

    with tile.TileContext(nc) as tc:
        with (
            tc.tile_pool(name="xp", bufs=6) as xp,
            tc.tile_pool(name="ep", bufs=1) as ep,
            tc.tile_pool(name="st", bufs=1) as st,
        ):
            # separate main/tail accumulator tiles so the early out DMA
            # (reads main) has no WAR hazard against the tail accums
            sums_main = st.tile([P, MAIN_COLS], _F32)
            sums_tail = st.tile([P, TAIL_CHUNKS], _F32)

            for k in range(N_TILES - 1):
                xt = xp.tile([P, C], _F32)
                nc.sync.dma_start(out=xt[:], in_=x_ext[k * P:(k + 1) * P, :])
                et = ep.tile([P, C], _F8)
                nc.scalar.activation(
                    out=et[:], in_=xt[:], func=AF.Exp, scale=S,
                    accum_out=sums_main[:, k:k + 1])

            # main row-sum partials out; completes under the endgame
            nc.scalar.dma_start(out=out_ext[:, :MAIN_COLS], in_=sums_main[:])

            k = N_TILES - 1
            for j in range(TAIL_CHUNKS):
                xt = xp.tile([P, TAIL_CHUNK], _F32)
                nc.sync.dma_start(
                    out=xt[:],
                    in_=x_ext[k * P:(k + 1) * P,
                              j * TAIL_CHUNK:(j + 1) * TAIL_CHUNK])
                et = ep.tile([P, TAIL_CHUNK], _F8)
                nc.scalar.activation(
                    out=et[:], in_=xt[:], func=AF.Exp, scale=S,
                    accum_out=sums_tail[:, j:j + 1])

            nc.scalar.dma_start(out=out_ext[:, MAIN_COLS:], in_=sums_tail[:])

    nc.finalize()
    return nc


_NC_CACHE = {}


def _get_nc():
    if "nc" not in _NC_CACHE:
        _NC_CACHE["nc"] = build()
    return _NC_CACHE["nc"]


def make_in_maps(cls_score):
    cls_score = np.ascontiguousarray(np.asarray(cls_score, dtype=np.float32))
    return [
        {"cls_score": cls_score[i * N_SHARD:(i + 1) * N_SHARD]}
        for i in range(N_CORES)
    ]


def postprocess(results, cls_score, labels):
    """Host epilogue in float64 off the device per-row exp-sums."""
    cls_score = np.asarray(cls_score, dtype=np.float32)
    labels = np.asarray(labels).astype(np.int64)
    rowsum = np.empty((N,), dtype=np.float64)
    for i, r in enumerate(results):
        o = r["out"].astype(np.float64)                    # [P, OUT_COLS]
        main = o[:, :MAIN_COLS]
        tailv = o[:, MAIN_COLS:].sum(axis=1)               # [P]
        # shard row n = k*P + p  ->  main[p, k] (k < 31) or tailv[p]
        rowsum[i * N_SHARD:(i + 1) * N_SHARD] = np.concatenate(
            [main.T.reshape(-1), tailv])
    target = cls_score[np.arange(N), labels].astype(np.float64)
    t = np.clip(target, -1.0 + EPS, 1.0 - EPS)
    num = S * np.cos(np.arccos(t) + M)
    excl = rowsum - np.exp(S * target)
    L = num - np.log(np.exp(num) + excl)
    return np.float32(-np.mean(L))


def kernel(cls_score, labels):
    nc = _get_nc()
    in_maps = make_in_maps(cls_score)
    res = run_bass_kernel_spmd(nc, in_maps, core_ids=list(range(N_CORES)))
    return postprocess(res.results, cls_score, labels)
